# revision 3
# baseline (speedup 1.0000x reference)
"""Branched feed-forward (4-phase MoE-style FF) on 8 Trainium2 NeuronCores.

Reference computation (B=32, S=1024, D=1024, P=4, F=4096):
    xs = x.reshape(B, P, S//P, D)              # static contiguous phase split
    h  = relu(xs @ W1[p] + b1[p])              # per-phase FF, D -> F
    y  = h @ W2[p] + b2[p]                     # F -> D
    out = y.reshape(B, S, D)

Sharding v2: 8 cores = 4 phases x 2 batch-halves.  Core c handles phase
p = c//2 and batches [16*(c%2), 16*(c%2+1)): 4096 tokens through the FULL
phase FF (no cross-core reduction; b1 and b2 both added on device, host
does pure reshape/transpose).

Per-core kernel (weights SBUF-resident, bf16 matmuls, fp32 psum):
    for each token block (TT tokens):
        FF1: h[ft, :] = relu( sum_dc W1[dc,ft].T @ xT[dc, :] + b1[ft] )
        FF2: y[dt, :] = sum_fc W2[fc,dt].T @ h[fc, :] + b2[dt]
"""

import numpy as np

import concourse.bacc as bacc
import concourse.mybir as mybir
import concourse.tile as tile
from concourse.bass import ts

# Problem dims (hardcoded per contest contract)
B, S, D = 32, 1024, 1024
P, F = 4, 4096
N_CORES = 8

# Per-core dims
BH = B // 2          # batches per core = 16
T2 = BH * (S // P)   # tokens per core = 4096
DC = D // 128        # 8 contraction chunks for FF1 / out tiles for FF2
FT = F // 128        # 32 out tiles for FF1 / contraction chunks for FF2

# Tunables (defaults = the graded configuration)
MM_DT = "bfloat16"   # matmul dtype
TT = 256             # token block (matmul moving free dim)

F32 = mybir.dt.float32


def build_bass(reps=1, loop_reps=1, mm_dt=None, tt=None):
    """Build the per-core Bass program (see module docstring)."""
    mm_dt = MM_DT if mm_dt is None else mm_dt
    tt = TT if tt is None else tt
    DT = getattr(mybir.dt, mm_dt)
    tb_n = T2 // tt

    # SBUF budget (~207.8 KB/partition usable): weights + h + x + y tiles
    esz = mybir.dt.size(DT)
    w_bytes = (DC * F + FT * D) * esz
    h_bytes = FT * tt * esz
    x_bytes = DC * tt * esz
    y_bytes = tt * 4
    budget = 204 * 1024
    h_bufs = 2
    x_bufs = 4
    y_bufs = 4
    while w_bytes + h_bufs * h_bytes + x_bufs * x_bytes + y_bufs * y_bytes + 512 > budget:
        if x_bufs > 2:
            x_bufs -= 1
        elif h_bufs > 1:
            h_bufs -= 1
        else:
            break

    nc = bacc.Bacc(None, target_bir_lowering=False)

    # Host pre-permutes everything so every DMA line is one contiguous
    # per-partition chunk.
    x_d = nc.dram_tensor("x", [tb_n, 128, DC, tt], DT, kind="ExternalInput")
    w1_d = nc.dram_tensor("w1", [128, DC, F], DT, kind="ExternalInput")
    w2_d = nc.dram_tensor("w2", [128, FT, D], DT, kind="ExternalInput")
    b1_d = nc.dram_tensor("b1", [128, FT], F32, kind="ExternalInput")
    b2_d = nc.dram_tensor("b2", [128, DC], F32, kind="ExternalInput")
    YDT = mybir.dt.bfloat16
    y_d = nc.dram_tensor("y", [tb_n, DC, 128, tt], YDT, kind="ExternalOutput")

    with tile.TileContext(nc) as tc:
        with (
            tc.tile_pool(name="weights", bufs=1) as wpool,
            tc.tile_pool(name="xin", bufs=x_bufs) as xpool,
            tc.tile_pool(name="hbuf", bufs=h_bufs) as hpool,
            tc.tile_pool(name="yout", bufs=y_bufs) as ypool,
            tc.tile_pool(name="psum", bufs=8, space="PSUM") as psum,
        ):
            w1_s = wpool.tile([128, DC, F], DT)
            nc.sync.dma_start(w1_s[:], w1_d[:])
            w2_s = wpool.tile([128, FT, D], DT)
            nc.sync.dma_start(w2_s[:], w2_d[:])
            b1_s = wpool.tile([128, FT], F32)
            nc.sync.dma_start(b1_s[:], b1_d[:])
            b2_s = wpool.tile([128, DC], F32)
            nc.sync.dma_start(b2_s[:], b2_d[:])

            def sweep():
                for tb in [t for _ in range(reps) for t in range(tb_n)]:
                    x_t = xpool.tile([128, DC, tt], DT, tag="x")
                    nc.sync.dma_start(x_t[:], x_d[tb])

                    h_t = hpool.tile([128, FT, tt], DT, tag="h")
                    for ft in range(FT):
                        ps = psum.tile([128, tt], F32, tag="ps")
                        for dc in range(DC):
                            nc.tensor.matmul(
                                ps[:],
                                w1_s[:, dc, ts(ft, 128)],
                                x_t[:, dc, :],
                                start=(dc == 0),
                                stop=(dc == DC - 1),
                            )
                        nc.scalar.activation(
                            h_t[:, ft, :],
                            ps[:],
                            mybir.ActivationFunctionType.Relu,
                            bias=b1_s[:, ft : ft + 1],
                        )

                    for dt_ in range(DC):
                        ps = psum.tile([128, tt], F32, tag="ps")
                        for fc in range(FT):
                            nc.tensor.matmul(
                                ps[:],
                                w2_s[:, fc, ts(dt_, 128)],
                                h_t[:, fc, :],
                                start=(fc == 0),
                                stop=(fc == FT - 1),
                            )
                        y_t = ypool.tile([128, tt], YDT, tag="y")
                        nc.scalar.add(y_t[:], ps[:], b2_s[:, dt_ : dt_ + 1])
                        nc.sync.dma_start(y_d[tb, dt_], y_t[:])

            if loop_reps > 1:
                with tc.For_i(0, loop_reps, 1):
                    sweep()
            else:
                sweep()

    nc.compile()
    return nc


def _np_dt(mm_dt=None):
    return mybir.dt.np(getattr(mybir.dt, MM_DT if mm_dt is None else mm_dt))


def _shard_inputs(x, W1, b1, W2, b2, mm_dt=None, tt=None):
    """Build the 8 per-core input maps. Core c: phase c//2, batch-half c%2."""
    tt = TT if tt is None else tt
    tb_n = T2 // tt
    np_dt = _np_dt(mm_dt)
    xs = x.reshape(B, P, S // P, D)
    in_maps = []
    w_by_phase = {}
    for c in range(N_CORES):
        p, bh = divmod(c, 2)
        if p not in w_by_phase:  # both batch-half cores of a phase share W
            w1 = np.ascontiguousarray(
                W1[p].reshape(DC, 128, F).transpose(1, 0, 2)
            ).astype(np_dt)
            w2 = np.ascontiguousarray(
                W2[p].reshape(FT, 128, D).transpose(1, 0, 2)
            ).astype(np_dt)
            b1c = np.ascontiguousarray(b1[p].reshape(FT, 128).T).astype(np.float32)
            b2c = np.ascontiguousarray(b2[p].reshape(DC, 128).T).astype(np.float32)
            w_by_phase[p] = (w1, w2, b1c, b2c)
        w1, w2, b1c, b2c = w_by_phase[p]
        xp = xs[bh * BH : (bh + 1) * BH, p]  # [BH, S//P, D]
        xt = np.ascontiguousarray(
            xp.reshape(tb_n, tt, DC, 128).transpose(0, 3, 2, 1)  # [tbn,128,DC,tt]
        ).astype(np_dt)
        in_maps.append({"x": xt, "w1": w1, "w2": w2, "b1": b1c, "b2": b2c})
    return in_maps


def _unshard_outputs(results, tt=None):
    """results: list of 8 dicts with 'y' [tb_n,DC,128,tt] final outputs."""
    tt = TT if tt is None else tt
    y = np.empty((B, P, S // P, D), dtype=np.float32)
    for c in range(N_CORES):
        p, bh = divmod(c, 2)
        yc = results[c]["y"]
        # [tbn,DC,128,tt] -> [tbn,tt,DC,128] -> [T2, D]
        yp = np.ascontiguousarray(yc.astype(np.float32).transpose(0, 3, 1, 2)).reshape(T2, D)
        y[bh * BH : (bh + 1) * BH, p] = yp.reshape(BH, S // P, D)
    return y.reshape(B, S, D)


# ---------------------------------------------------------------------------
# Compile-once PJRT runner (caches the sharded executable so repeat kernel()
# calls skip re-tracing).

_RUNNER = None


def _make_runner():
    import jax
    from jax.sharding import Mesh, PartitionSpec
    from jax.experimental.shard_map import shard_map
    from concourse.bass2jax import (
        _bass_exec_p,
        install_neuronx_cc_hook,
        partition_id_tensor,
    )

    nc = build_bass()
    install_neuronx_cc_hook()

    partition_name = nc.partition_id_tensor.name if nc.partition_id_tensor else None

    in_names, out_names, out_avals = [], [], []
    for alloc in nc.m.functions[0].allocations:
        if not isinstance(alloc, mybir.MemoryLocationSet):
            continue
        name = alloc.memorylocations[0].name
        if alloc.kind == "ExternalInput":
            if name != partition_name:
                in_names.append(name)
        elif alloc.kind == "ExternalOutput":
            out_names.append(name)
            out_avals.append(
                jax.core.ShapedArray(
                    tuple(alloc.tensor_shape), mybir.dt.np(alloc.dtype)
                )
            )
    n_params = len(in_names)
    all_in_names = list(in_names) + list(out_names)
    if partition_name is not None:
        all_in_names.append(partition_name)

    def _body(*args):
        operands = list(args)
        if partition_name is not None:
            operands.append(partition_id_tensor())
        outs = _bass_exec_p.bind(
            *operands,
            out_avals=tuple(out_avals),
            in_names=tuple(all_in_names),
            out_names=tuple(out_names),
            lowering_input_output_aliases=(),
            sim_require_finite=True,
            sim_require_nnan=True,
            nc=nc,
        )
        return tuple(outs)

    devices = jax.devices()[:N_CORES]
    mesh = Mesh(np.asarray(devices), ("core",))
    n_outs = len(out_names)
    jitted = jax.jit(
        shard_map(
            _body,
            mesh=mesh,
            in_specs=(PartitionSpec("core"),) * (n_params + n_outs),
            out_specs=(PartitionSpec("core"),) * n_outs,
            check_rep=False,
        ),
        keep_unused=True,
    )

    def run(in_maps):
        concat_in = [
            np.concatenate(
                [np.asarray(in_maps[c][nm]) for c in range(N_CORES)], axis=0
            )
            for nm in in_names
        ]
        concat_zeros = [
            np.zeros((N_CORES * a.shape[0], *a.shape[1:]), a.dtype)
            for a in out_avals
        ]
        outs = jitted(*concat_in, *concat_zeros)
        return [
            {
                nm: np.asarray(outs[i]).reshape(N_CORES, *out_avals[i].shape)[c]
                for i, nm in enumerate(out_names)
            }
            for c in range(N_CORES)
        ]

    return run


def kernel(x, W1, b1, W2, b2, phases):
    """Full-input entry point. `phases` is unused: the reference's phase
    assignment is the static contiguous partition of the sequence."""
    global _RUNNER
    x = np.asarray(x, dtype=np.float32)
    W1 = np.asarray(W1, dtype=np.float32)
    b1 = np.asarray(b1, dtype=np.float32)
    W2 = np.asarray(W2, dtype=np.float32)
    b2 = np.asarray(b2, dtype=np.float32)

    if _RUNNER is None:
        _RUNNER = _make_runner()
    in_maps = _shard_inputs(x, W1, b1, W2, b2)
    try:
        results = _RUNNER(in_maps)
    except Exception:
        # transient NRT device errors have been observed; retry once
        results = _RUNNER(in_maps)
    return _unshard_outputs(results)


if __name__ == "__main__":
    rng = np.random.default_rng(0)
    x = rng.standard_normal((B, S, D), dtype=np.float32)
    W1 = (rng.random((P, D, F), dtype=np.float32) - 0.5) / np.sqrt(D)
    b1 = (rng.random((P, F), dtype=np.float32) - 0.5) / np.sqrt(D)
    W2 = (rng.random((P, F, D), dtype=np.float32) - 0.5) / np.sqrt(F)
    b2 = (rng.random((P, D), dtype=np.float32) - 0.5) / np.sqrt(F)
    phases = rng.integers(0, P, size=(B, S)).astype(np.int32)

    y = kernel(x, W1, b1, W2, b2, phases)

    xs = x.reshape(B, P, S // P, D)
    h = np.maximum(np.einsum("bpsd,pdf->bpsf", xs, W1) + b1[None, :, None, :], 0.0)
    yref = (np.einsum("bpsf,pfd->bpsd", h, W2) + b2[None, :, None, :]).reshape(B, S, D)
    err = np.linalg.norm(y - yref) / np.linalg.norm(yref)
    print("rel err:", err)


# revision 4
# speedup vs baseline: 1.0106x; 1.0106x over previous
"""Branched feed-forward (4-phase MoE-style FF) on 8 Trainium2 NeuronCores.

Reference computation (B=32, S=1024, D=1024, P=4, F=4096):
    xs = x.reshape(B, P, S//P, D)              # static contiguous phase split
    h  = relu(xs @ W1[p] + b1[p])              # per-phase FF, D -> F
    y  = h @ W2[p] + b2[p]                     # F -> D
    out = y.reshape(B, S, D)

Sharding v2: 8 cores = 4 phases x 2 batch-halves.  Core c handles phase
p = c//2 and batches [16*(c%2), 16*(c%2+1)): 4096 tokens through the FULL
phase FF (no cross-core reduction; b1 and b2 both added on device, host
does pure reshape/transpose).

Per-core kernel (weights SBUF-resident, bf16 matmuls, fp32 psum):
    for each token block (TT tokens):
        FF1: h[ft, :] = relu( sum_dc W1[dc,ft].T @ xT[dc, :] + b1[ft] )
        FF2: y[dt, :] = sum_fc W2[fc,dt].T @ h[fc, :] + b2[dt]
"""

import numpy as np

import concourse.bacc as bacc
import concourse.mybir as mybir
import concourse.tile as tile
from concourse.bass import ts

# Problem dims (hardcoded per contest contract)
B, S, D = 32, 1024, 1024
P, F = 4, 4096
N_CORES = 8

# Per-core dims
BH = B // 2          # batches per core = 16
T2 = BH * (S // P)   # tokens per core = 4096
DC = D // 128        # 8 contraction chunks for FF1 / out tiles for FF2
FT = F // 128        # 32 out tiles for FF1 / contraction chunks for FF2

# Tunables (defaults = the graded configuration)
MM_DT = "bfloat16"   # matmul dtype
TT = 512             # token block (matmul moving free dim)

F32 = mybir.dt.float32


def build_bass(reps=1, loop_reps=1, mm_dt=None, tt=None):
    """Build the per-core Bass program (see module docstring)."""
    mm_dt = MM_DT if mm_dt is None else mm_dt
    tt = TT if tt is None else tt
    DT = getattr(mybir.dt, mm_dt)
    tb_n = T2 // tt

    # SBUF budget (~207.8 KB/partition usable): weights + h + x + y tiles
    esz = mybir.dt.size(DT)
    w_bytes = (DC * F + FT * D) * esz
    h_bytes = FT * tt * esz
    x_bytes = DC * tt * esz
    y_bytes = tt * 4
    budget = 204 * 1024
    h_bufs = 2
    x_bufs = 4
    y_bufs = 4
    while w_bytes + h_bufs * h_bytes + x_bufs * x_bytes + y_bufs * y_bytes + 512 > budget:
        if x_bufs > 2:
            x_bufs -= 1
        elif h_bufs > 1:
            h_bufs -= 1
        else:
            break

    nc = bacc.Bacc(None, target_bir_lowering=False)

    # Host pre-permutes everything so every DMA line is one contiguous
    # per-partition chunk.
    x_d = nc.dram_tensor("x", [tb_n, 128, DC, tt], DT, kind="ExternalInput")
    w1_d = nc.dram_tensor("w1", [128, DC, F], DT, kind="ExternalInput")
    w2_d = nc.dram_tensor("w2", [128, FT, D], DT, kind="ExternalInput")
    b1_d = nc.dram_tensor("b1", [128, FT], F32, kind="ExternalInput")
    b2_d = nc.dram_tensor("b2", [128, DC], F32, kind="ExternalInput")
    YDT = mybir.dt.bfloat16
    y_d = nc.dram_tensor("y", [tb_n, DC, 128, tt], YDT, kind="ExternalOutput")

    with tile.TileContext(nc) as tc:
        with (
            tc.tile_pool(name="weights", bufs=1) as wpool,
            tc.tile_pool(name="xin", bufs=x_bufs) as xpool,
            tc.tile_pool(name="hbuf", bufs=h_bufs) as hpool,
            tc.tile_pool(name="yout", bufs=y_bufs) as ypool,
            tc.tile_pool(name="psum", bufs=8, space="PSUM") as psum,
        ):
            w1_s = wpool.tile([128, DC, F], DT)
            nc.sync.dma_start(w1_s[:], w1_d[:])
            w2_s = wpool.tile([128, FT, D], DT)
            nc.sync.dma_start(w2_s[:], w2_d[:])
            b1_s = wpool.tile([128, FT], F32)
            nc.sync.dma_start(b1_s[:], b1_d[:])
            b2_s = wpool.tile([128, DC], F32)
            nc.sync.dma_start(b2_s[:], b2_d[:])

            def sweep():
                for tb in [t for _ in range(reps) for t in range(tb_n)]:
                    x_t = xpool.tile([128, DC, tt], DT, tag="x")
                    nc.sync.dma_start(x_t[:], x_d[tb])

                    h_t = hpool.tile([128, FT, tt], DT, tag="h")
                    for ft in range(FT):
                        ps = psum.tile([128, tt], F32, tag="ps")
                        for dc in range(DC):
                            nc.tensor.matmul(
                                ps[:],
                                w1_s[:, dc, ts(ft, 128)],
                                x_t[:, dc, :],
                                start=(dc == 0),
                                stop=(dc == DC - 1),
                            )
                        nc.scalar.activation(
                            h_t[:, ft, :],
                            ps[:],
                            mybir.ActivationFunctionType.Relu,
                            bias=b1_s[:, ft : ft + 1],
                        )

                    for dt_ in range(DC):
                        ps = psum.tile([128, tt], F32, tag="ps")
                        for fc in range(FT):
                            nc.tensor.matmul(
                                ps[:],
                                w2_s[:, fc, ts(dt_, 128)],
                                h_t[:, fc, :],
                                start=(fc == 0),
                                stop=(fc == FT - 1),
                            )
                        y_t = ypool.tile([128, tt], YDT, tag="y")
                        nc.scalar.add(y_t[:], ps[:], b2_s[:, dt_ : dt_ + 1])
                        nc.sync.dma_start(y_d[tb, dt_], y_t[:])

            if loop_reps > 1:
                with tc.For_i(0, loop_reps, 1):
                    sweep()
            else:
                sweep()

    nc.compile()
    return nc


def _np_dt(mm_dt=None):
    return mybir.dt.np(getattr(mybir.dt, MM_DT if mm_dt is None else mm_dt))


def _shard_inputs(x, W1, b1, W2, b2, mm_dt=None, tt=None):
    """Build the 8 per-core input maps. Core c: phase c//2, batch-half c%2."""
    tt = TT if tt is None else tt
    tb_n = T2 // tt
    np_dt = _np_dt(mm_dt)
    xs = x.reshape(B, P, S // P, D)
    in_maps = []
    w_by_phase = {}
    for c in range(N_CORES):
        p, bh = divmod(c, 2)
        if p not in w_by_phase:  # both batch-half cores of a phase share W
            w1 = np.ascontiguousarray(
                W1[p].reshape(DC, 128, F).transpose(1, 0, 2)
            ).astype(np_dt)
            w2 = np.ascontiguousarray(
                W2[p].reshape(FT, 128, D).transpose(1, 0, 2)
            ).astype(np_dt)
            b1c = np.ascontiguousarray(b1[p].reshape(FT, 128).T).astype(np.float32)
            b2c = np.ascontiguousarray(b2[p].reshape(DC, 128).T).astype(np.float32)
            w_by_phase[p] = (w1, w2, b1c, b2c)
        w1, w2, b1c, b2c = w_by_phase[p]
        xp = xs[bh * BH : (bh + 1) * BH, p]  # [BH, S//P, D]
        xt = np.ascontiguousarray(
            xp.reshape(tb_n, tt, DC, 128).transpose(0, 3, 2, 1)  # [tbn,128,DC,tt]
        ).astype(np_dt)
        in_maps.append({"x": xt, "w1": w1, "w2": w2, "b1": b1c, "b2": b2c})
    return in_maps


def _unshard_outputs(results, tt=None):
    """results: list of 8 dicts with 'y' [tb_n,DC,128,tt] final outputs."""
    tt = TT if tt is None else tt
    y = np.empty((B, P, S // P, D), dtype=np.float32)
    for c in range(N_CORES):
        p, bh = divmod(c, 2)
        yc = results[c]["y"]
        # [tbn,DC,128,tt] -> [tbn,tt,DC,128] -> [T2, D]
        yp = np.ascontiguousarray(yc.astype(np.float32).transpose(0, 3, 1, 2)).reshape(T2, D)
        y[bh * BH : (bh + 1) * BH, p] = yp.reshape(BH, S // P, D)
    return y.reshape(B, S, D)


# ---------------------------------------------------------------------------
# Compile-once PJRT runner (caches the sharded executable so repeat kernel()
# calls skip re-tracing).

_RUNNER = None


def _make_runner():
    import jax
    from jax.sharding import Mesh, PartitionSpec
    from jax.experimental.shard_map import shard_map
    from concourse.bass2jax import (
        _bass_exec_p,
        install_neuronx_cc_hook,
        partition_id_tensor,
    )

    nc = build_bass()
    install_neuronx_cc_hook()

    partition_name = nc.partition_id_tensor.name if nc.partition_id_tensor else None

    in_names, out_names, out_avals = [], [], []
    for alloc in nc.m.functions[0].allocations:
        if not isinstance(alloc, mybir.MemoryLocationSet):
            continue
        name = alloc.memorylocations[0].name
        if alloc.kind == "ExternalInput":
            if name != partition_name:
                in_names.append(name)
        elif alloc.kind == "ExternalOutput":
            out_names.append(name)
            out_avals.append(
                jax.core.ShapedArray(
                    tuple(alloc.tensor_shape), mybir.dt.np(alloc.dtype)
                )
            )
    n_params = len(in_names)
    all_in_names = list(in_names) + list(out_names)
    if partition_name is not None:
        all_in_names.append(partition_name)

    def _body(*args):
        operands = list(args)
        if partition_name is not None:
            operands.append(partition_id_tensor())
        outs = _bass_exec_p.bind(
            *operands,
            out_avals=tuple(out_avals),
            in_names=tuple(all_in_names),
            out_names=tuple(out_names),
            lowering_input_output_aliases=(),
            sim_require_finite=True,
            sim_require_nnan=True,
            nc=nc,
        )
        return tuple(outs)

    devices = jax.devices()[:N_CORES]
    mesh = Mesh(np.asarray(devices), ("core",))
    n_outs = len(out_names)
    jitted = jax.jit(
        shard_map(
            _body,
            mesh=mesh,
            in_specs=(PartitionSpec("core"),) * (n_params + n_outs),
            out_specs=(PartitionSpec("core"),) * n_outs,
            check_rep=False,
        ),
        keep_unused=True,
    )

    def run(in_maps):
        concat_in = [
            np.concatenate(
                [np.asarray(in_maps[c][nm]) for c in range(N_CORES)], axis=0
            )
            for nm in in_names
        ]
        concat_zeros = [
            np.zeros((N_CORES * a.shape[0], *a.shape[1:]), a.dtype)
            for a in out_avals
        ]
        outs = jitted(*concat_in, *concat_zeros)
        return [
            {
                nm: np.asarray(outs[i]).reshape(N_CORES, *out_avals[i].shape)[c]
                for i, nm in enumerate(out_names)
            }
            for c in range(N_CORES)
        ]

    return run


def kernel(x, W1, b1, W2, b2, phases):
    """Full-input entry point. `phases` is unused: the reference's phase
    assignment is the static contiguous partition of the sequence."""
    global _RUNNER
    x = np.asarray(x, dtype=np.float32)
    W1 = np.asarray(W1, dtype=np.float32)
    b1 = np.asarray(b1, dtype=np.float32)
    W2 = np.asarray(W2, dtype=np.float32)
    b2 = np.asarray(b2, dtype=np.float32)

    if _RUNNER is None:
        _RUNNER = _make_runner()
    in_maps = _shard_inputs(x, W1, b1, W2, b2)
    try:
        results = _RUNNER(in_maps)
    except Exception:
        # transient NRT device errors have been observed; retry once
        results = _RUNNER(in_maps)
    return _unshard_outputs(results)


if __name__ == "__main__":
    rng = np.random.default_rng(0)
    x = rng.standard_normal((B, S, D), dtype=np.float32)
    W1 = (rng.random((P, D, F), dtype=np.float32) - 0.5) / np.sqrt(D)
    b1 = (rng.random((P, F), dtype=np.float32) - 0.5) / np.sqrt(D)
    W2 = (rng.random((P, F, D), dtype=np.float32) - 0.5) / np.sqrt(F)
    b2 = (rng.random((P, D), dtype=np.float32) - 0.5) / np.sqrt(F)
    phases = rng.integers(0, P, size=(B, S)).astype(np.int32)

    y = kernel(x, W1, b1, W2, b2, phases)

    xs = x.reshape(B, P, S // P, D)
    h = np.maximum(np.einsum("bpsd,pdf->bpsf", xs, W1) + b1[None, :, None, :], 0.0)
    yref = (np.einsum("bpsf,pfd->bpsd", h, W2) + b2[None, :, None, :]).reshape(B, S, D)
    err = np.linalg.norm(y - yref) / np.linalg.norm(yref)
    print("rel err:", err)
